# revision 33
# baseline (speedup 1.0000x reference)
"""DLRM dot-interaction kernel for Trainium2 (8 NeuronCores, batch-sharded).

Per sample b: T = concat(dense[b], embs[b]) -> [27, 128]; Z = T @ T^T;
output = strict upper triangle of Z -> [351] fp32.

Per-core plan (2048 samples, 16 blocks of 128):
  - SWDGE cast-DMA loads one block at a time (fp32 -> fp16), block 0 in
    two half-tiles so the PE can start transposing ~3us in.  Per-block
    loads keep input arrival smooth: the kernel is paced by the PE with
    the input stream just barely keeping up, so 4-block load lumps would
    directly stall the in-order PE queue.
  - PE transposes each [128 b, 128 d] feature slab (transpose-mode fp16,
    LDWEIGHTS+MM pair ~107ns at the HAM-throttled 1.2 GHz clock; the HAM
    never warms for transpose-mode work, so this is the steady rate).
  - Per-sample gram matmuls: lhsT = rhs = [128 d, 27 f] strided slice of
    f-major Tt; out -> PSUM zp[32*g + m, q*32 + n] fp32, col-group
    tiling (sample s = q*4 + g), ~34ns/sample (serial LDW+MM; the
    toolchain compiles with --enable-ldw-opt=false and bass emits
    standalone InstLdweights, so LDW/MM overlap is not available).
  - One DVE StreamTranspose per block swaps m<->q inside each quadrant:
    PSUM [(g,m), (n,q)-view] -> SBUF Zb[(g,q), m*32+n] fp32, replacing a
    DRAM scratch bounce + 55k-descriptor gather with 16 instructions.
  - Triu pack: 26 contiguous-run DVE/ACT copies per pack group into
    Pk [(g,q), t*351]; HWDGE DMAs with 1404B runs write out[b, :]
    (partition (g,q) -> row q*4+g).  The last two blocks pack singly to
    shorten the drain tail.
"""

import numpy as np

B, NUM_EMBS, D = 16384, 26, 128
N_CORES = 8
BC = B // N_CORES  # 2048 samples per core
BLK = 128          # samples per block
NF = NUM_EMBS + 1  # 27 features
FP = 32            # feature pitch in the Z PSUM tile
NPAIR = NF * (NF - 1) // 2  # 351

_CACHE = {}


def build(bc=BC):
    import concourse.bacc as bacc
    import concourse.mybir as mybir
    from concourse.tile import TileContext
    from concourse.masks import make_identity

    fp16 = mybir.dt.float16
    fp32 = mybir.dt.float32

    nc = bacc.Bacc("TRN2", target_bir_lowering=False, debug=False)
    dense_t = nc.dram_tensor("dense", (bc, D), fp32, kind="ExternalInput")
    embs_t = nc.dram_tensor("embs", (bc, NUM_EMBS, D), fp32, kind="ExternalInput")
    out_t = nc.dram_tensor("out", (bc, NPAIR), fp32, kind="ExternalOutput")

    nblk = bc // BLK
    # pack groups: pairs, with the final two blocks packed singly so the
    # post-PE drain (StreamTranspose + pack + out DMA) tail is short
    pgroups = [(b, b + 1) for b in range(0, nblk - 2, 2)] + [(nblk - 2,), (nblk - 1,)]
    pg_of = {}
    for gi, grp in enumerate(pgroups):
        for b in grp:
            pg_of[b] = (gi, grp)

    with TileContext(nc) as tc:
        with (
            tc.tile_pool(name="consts", bufs=1) as consts,
            tc.tile_pool(name="xin", bufs=7) as xpool,
            tc.tile_pool(name="tt", bufs=4) as ttpool,
            tc.tile_pool(name="zb", bufs=3) as zbpool,
            tc.tile_pool(name="pk", bufs=3) as pkpool,
            tc.tile_pool(name="tp", bufs=2, space="PSUM") as tppool,
            tc.tile_pool(name="zp", bufs=3, space="PSUM") as zppool,
        ):
            ident = consts.tile([128, 128], fp16)
            make_identity(nc, ident)

            dview = dense_t.ap()  # [bc, 128]
            eview = embs_t.ap().rearrange("b f d -> b (f d)")  # [bc, 3328]
            oview = out_t.ap()  # [bc, 351]

            xmap = {}   # blk -> list of (tile, f0, nf) segments
            tts = {}
            zps = {}
            zb_t = None
            pk_t = None

            def emit_load(blk):
                b0 = blk * BLK
                if blk == 0:
                    # two half-tiles so transposes start after ~half the load
                    X0 = xpool.tile([BLK, 14 * D], fp16, tag="Xa")
                    nc.gpsimd.dma_start(out=X0[:, 0:D], in_=dview[0:BLK])
                    nc.gpsimd.dma_start(
                        out=X0[:, D:], in_=eview[0:BLK, : 13 * D]
                    )
                    X1 = xpool.tile([BLK, 13 * D], fp16, tag="Xb")
                    nc.gpsimd.dma_start(out=X1[:, :], in_=eview[0:BLK, 13 * D :])
                    xmap[blk] = [(X0, 0, 14), (X1, 14, 13)]
                else:
                    X = xpool.tile([BLK, NF * D], fp16, tag="X")
                    nc.gpsimd.dma_start(
                        out=X[:, 0:D], in_=dview[b0 : b0 + BLK]
                    )
                    nc.gpsimd.dma_start(
                        out=X[:, D:], in_=eview[b0 : b0 + BLK]
                    )
                    xmap[blk] = [(X, 0, NF)]

            def _slab(blk, f):
                for tile, f0, nf in xmap[blk]:
                    if f0 <= f < f0 + nf:
                        c0 = (f - f0) * D
                        return tile[:, c0 : c0 + D]
                raise AssertionError

            def emit_transpose_half(blk, half):
                """Half of the 27 b->d feature-slab transposes for blk."""
                if half == 0:
                    Tt = ttpool.tile([128, NF * D], fp16, tag="Tt")
                    tts[blk] = Tt
                Tt = tts[blk]
                for ci in (0, 1) if half == 0 else (2, 3):
                    c0 = ci * 7
                    cf = min(7, NF - c0)
                    tp = tppool.tile([128, 7 * BLK], fp16, tag="tp")
                    for j in range(cf):
                        nc.tensor.transpose(
                            tp[:, j * BLK : (j + 1) * BLK],
                            _slab(blk, c0 + j),
                            ident,
                        )
                    dst = Tt[:, c0 * BLK : (c0 + cf) * BLK]
                    src = tp[:, : cf * BLK]
                    if ci % 2 == 0:
                        nc.vector.tensor_copy(out=dst, in_=src)
                    else:
                        nc.scalar.copy(dst, src)
                if half == 1:
                    del xmap[blk]

            def emit_gram_half(blk, half):
                """Half (64 samples) of the per-sample gram matmuls; the
                second half is followed by the DVE StreamTranspose."""
                Tt = tts[blk]
                Ttr = Tt.rearrange("d (f b) -> d b f", b=BLK)
                if half == 0:
                    zp = zppool.tile([128, FP * FP], fp32, tag="zp")
                    zps[blk] = zp
                zp = zps[blk]
                for q in range(16 * half, 16 * half + 16):
                    for g in range(4):
                        s = q * 4 + g
                        nc.tensor.matmul(
                            zp[32 * g : 32 * g + NF, q * FP : q * FP + NF],
                            Ttr[:, s, :],
                            Ttr[:, s, :],
                            start=True,
                            stop=True,
                            tile_position=(0, 32 * g),
                        )
                if half == 1:
                    del tts[blk]
                    gi, grp = pg_of[blk]
                    t = grp.index(blk)
                    zpt = zps.pop(blk)
                    inv = zpt.rearrange("p (q n) -> p n q", n=FP)[:, 0:NF, :]
                    outv = zb_t.rearrange(
                        "p (t m n) -> p t n m", t=len(grp), n=FP
                    )[:, t, 0:NF, :]
                    nc.vector.transpose(out=outv, in_=inv)

            def gram_pre(blk):
                nonlocal zb_t
                gi, grp = pg_of[blk]
                if blk == grp[0]:
                    zb_t = zbpool.tile(
                        [128, len(grp) * FP * FP], fp32, tag="Zb"
                    )

            def gram_post(blk):
                gi, grp = pg_of[blk]
                if blk != grp[-1]:
                    return
                npk = len(grp)
                zbp = zb_t.rearrange("p (t m n) -> p t m n", t=npk, n=FP)
                Pk = pkpool.tile([128, npk * NPAIR], fp32, tag="Pk")
                pkp = Pk.rearrange("p (t c) -> p t c", t=npk)
                off = 0
                for m in range(NF - 1):
                    ln = NF - 1 - m
                    src = zbp[:, :, m, m + 1 : NF]
                    dst = pkp[:, :, off : off + ln]
                    if m % 2 == 0:
                        nc.vector.tensor_copy(out=dst, in_=src)
                    else:
                        nc.scalar.copy(dst, src)
                    off += ln
                b0 = grp[0] * BLK
                ovq = oview[b0 : b0 + npk * BLK].rearrange(
                    "(t q g) c -> g q t c", t=npk, g=4
                )
                pk4 = pkp.rearrange("(g q) t c -> g q t c", g=4)
                for g in range(4):
                    eng = nc.sync if g % 2 == 0 else nc.scalar
                    eng.dma_start(out=ovq[g], in_=pk4[g])

            # Pipeline: loads prefetch 2 blocks ahead of the transposes;
            # gram matmuls lag the transposes by one block so the PE queue
            # always has a gram burst banked behind a possibly-input-
            # stalled transpose burst.
            def emit_gram_block(blk):
                gram_pre(blk)
                emit_gram_half(blk, 0)
                emit_gram_half(blk, 1)
                gram_post(blk)

            for blk in range(5):
                emit_load(blk)
            emit_transpose_half(0, 0)
            emit_transpose_half(0, 1)
            emit_transpose_half(1, 0)
            emit_transpose_half(1, 1)
            emit_gram_block(0)
            emit_gram_block(1)
            for blk in range(2, nblk):
                if blk + 3 < nblk:
                    emit_load(blk + 3)
                emit_transpose_half(blk, 0)
                emit_transpose_half(blk, 1)
                if blk >= 3:
                    emit_gram_block(blk - 1)
            emit_gram_block(nblk - 1)

    nc.compile()
    return nc


def _get(bc=BC):
    if bc not in _CACHE:
        _CACHE[bc] = build(bc)
    return _CACHE[bc]


def kernel(dense: np.ndarray, embs: np.ndarray) -> np.ndarray:
    from concourse import bass_utils

    dense = np.ascontiguousarray(np.asarray(dense, dtype=np.float32))
    embs = np.ascontiguousarray(np.asarray(embs, dtype=np.float32))
    assert dense.shape == (B, D) and embs.shape == (B, NUM_EMBS, D)

    nc = _get()
    dsh = dense.reshape(N_CORES, BC, D)
    esh = embs.reshape(N_CORES, BC, NUM_EMBS, D)
    in_maps = [{"dense": dsh[i], "embs": esh[i]} for i in range(N_CORES)]
    res = bass_utils.run_bass_kernel_spmd(nc, in_maps, core_ids=list(range(N_CORES)))
    return np.concatenate([r["out"] for r in res.results], axis=0)


# revision 34
# speedup vs baseline: 1.0753x; 1.0753x over previous
"""DLRM dot-interaction kernel for Trainium2 (8 NeuronCores, batch-sharded).

Per sample b: T = concat(dense[b], embs[b]) -> [27, 128]; Z = T @ T^T;
output = strict upper triangle of Z -> [351] fp32.

Per-core plan (2048 samples, 16 blocks of 128):
  - SWDGE cast-DMA loads one block at a time (fp32 -> fp16), block 0 in
    two half-tiles so the PE can start transposing ~3us in.  Per-block
    loads keep input arrival smooth: the kernel is paced by the PE with
    the input stream just barely keeping up, so 4-block load lumps would
    directly stall the in-order PE queue.
  - PE transposes each [128 b, 128 d] feature slab (transpose-mode fp16,
    LDWEIGHTS+MM pair ~107ns at the HAM-throttled 1.2 GHz clock; the HAM
    never warms for transpose-mode work, so this is the steady rate).
  - Per-sample gram matmuls: lhsT = rhs = [128 d, 27 f] strided slice of
    f-major Tt; out -> PSUM zp[32*g + m, q*32 + n] fp32, col-group
    tiling (sample s = q*4 + g), ~34ns/sample (serial LDW+MM; the
    toolchain compiles with --enable-ldw-opt=false and bass emits
    standalone InstLdweights, so LDW/MM overlap is not available).
  - One DVE StreamTranspose per block swaps m<->q inside each quadrant:
    PSUM [(g,m), (n,q)-view] -> SBUF Zb[(g,q), m*32+n] fp32, replacing a
    DRAM scratch bounce + 55k-descriptor gather with 16 instructions.
  - Triu pack: 26 contiguous-run DVE/ACT copies per pack group into
    Pk [(g,q), t*351]; HWDGE DMAs with 1404B runs write out[b, :]
    (partition (g,q) -> row q*4+g).  The last two blocks pack singly to
    shorten the drain tail.
"""

import numpy as np

B, NUM_EMBS, D = 16384, 26, 128
N_CORES = 8
BC = B // N_CORES  # 2048 samples per core
BLK = 128          # samples per block
NF = NUM_EMBS + 1  # 27 features
FP = 32            # feature pitch in the Z PSUM tile
NPAIR = NF * (NF - 1) // 2  # 351

_CACHE = {}


def build(bc=BC):
    import concourse.bacc as bacc
    import concourse.mybir as mybir
    from concourse.tile import TileContext
    from concourse.masks import make_identity

    fp16 = mybir.dt.float16
    fp32 = mybir.dt.float32

    nc = bacc.Bacc("TRN2", target_bir_lowering=False, debug=False)
    dense_t = nc.dram_tensor("dense", (bc, D), fp32, kind="ExternalInput")
    embs_t = nc.dram_tensor("embs", (bc, NUM_EMBS, D), fp32, kind="ExternalInput")
    out_t = nc.dram_tensor("out", (bc, NPAIR), fp32, kind="ExternalOutput")

    nblk = bc // BLK
    # pack groups: pairs, with the final two blocks packed singly so the
    # post-PE drain (StreamTranspose + pack + out DMA) tail is short
    pgroups = [(b, b + 1) for b in range(0, nblk - 2, 2)] + [(nblk - 2,), (nblk - 1,)]
    pg_of = {}
    for gi, grp in enumerate(pgroups):
        for b in grp:
            pg_of[b] = (gi, grp)

    with TileContext(nc) as tc:
        with (
            tc.tile_pool(name="consts", bufs=1) as consts,
            tc.tile_pool(name="xin", bufs=7) as xpool,
            tc.tile_pool(name="tt", bufs=4) as ttpool,
            tc.tile_pool(name="zb", bufs=3) as zbpool,
            tc.tile_pool(name="pk", bufs=3) as pkpool,
            tc.tile_pool(name="tp", bufs=4, space="PSUM") as tppool,
            tc.tile_pool(name="zp", bufs=2, space="PSUM") as zppool,
        ):
            ident = consts.tile([128, 128], fp16)
            make_identity(nc, ident)

            dview = dense_t.ap()  # [bc, 128]
            eview = embs_t.ap().rearrange("b f d -> b (f d)")  # [bc, 3328]
            oview = out_t.ap()  # [bc, 351]

            xmap = {}   # blk -> list of (tile, f0, nf) segments
            tts = {}
            zps = {}
            zb_t = None
            pk_t = None

            def emit_load(blk):
                b0 = blk * BLK
                if blk == 0:
                    # two half-tiles so transposes start after ~half the load
                    X0 = xpool.tile([BLK, 14 * D], fp16, tag="Xa")
                    nc.gpsimd.dma_start(out=X0[:, 0:D], in_=dview[0:BLK])
                    nc.gpsimd.dma_start(
                        out=X0[:, D:], in_=eview[0:BLK, : 13 * D]
                    )
                    X1 = xpool.tile([BLK, 13 * D], fp16, tag="Xb")
                    nc.gpsimd.dma_start(out=X1[:, :], in_=eview[0:BLK, 13 * D :])
                    xmap[blk] = [(X0, 0, 14), (X1, 14, 13)]
                else:
                    X = xpool.tile([BLK, NF * D], fp16, tag="X")
                    nc.gpsimd.dma_start(
                        out=X[:, 0:D], in_=dview[b0 : b0 + BLK]
                    )
                    nc.gpsimd.dma_start(
                        out=X[:, D:], in_=eview[b0 : b0 + BLK]
                    )
                    xmap[blk] = [(X, 0, NF)]

            def _slab(blk, f):
                for tile, f0, nf in xmap[blk]:
                    if f0 <= f < f0 + nf:
                        c0 = (f - f0) * D
                        return tile[:, c0 : c0 + D]
                raise AssertionError

            def emit_transpose_half(blk, half):
                """Half of the 27 b->d feature-slab transposes for blk."""
                if half == 0:
                    Tt = ttpool.tile([128, NF * D], fp16, tag="Tt")
                    tts[blk] = Tt
                Tt = tts[blk]
                for ci in (0, 1) if half == 0 else (2, 3):
                    c0 = ci * 7
                    cf = min(7, NF - c0)
                    tp = tppool.tile([128, 7 * BLK], fp16, tag="tp")
                    for j in range(cf):
                        nc.tensor.transpose(
                            tp[:, j * BLK : (j + 1) * BLK],
                            _slab(blk, c0 + j),
                            ident,
                        )
                    dst = Tt[:, c0 * BLK : (c0 + cf) * BLK]
                    src = tp[:, : cf * BLK]
                    if ci % 2 == 0:
                        nc.vector.tensor_copy(out=dst, in_=src)
                    else:
                        nc.scalar.copy(dst, src)
                if half == 1:
                    del xmap[blk]

            def emit_gram_half(blk, half):
                """Half (64 samples) of the per-sample gram matmuls; the
                second half is followed by the DVE StreamTranspose."""
                Tt = tts[blk]
                Ttr = Tt.rearrange("d (f b) -> d b f", b=BLK)
                if half == 0:
                    zp = zppool.tile([128, FP * FP], fp32, tag="zp")
                    zps[blk] = zp
                zp = zps[blk]
                for q in range(16 * half, 16 * half + 16):
                    for g in range(4):
                        s = q * 4 + g
                        nc.tensor.matmul(
                            zp[32 * g : 32 * g + NF, q * FP : q * FP + NF],
                            Ttr[:, s, :],
                            Ttr[:, s, :],
                            start=True,
                            stop=True,
                            tile_position=(0, 32 * g),
                        )
                if half == 1:
                    del tts[blk]
                    gi, grp = pg_of[blk]
                    t = grp.index(blk)
                    zpt = zps.pop(blk)
                    inv = zpt.rearrange("p (q n) -> p n q", n=FP)[:, 0:NF, :]
                    outv = zb_t.rearrange(
                        "p (t m n) -> p t n m", t=len(grp), n=FP
                    )[:, t, 0:NF, :]
                    nc.vector.transpose(out=outv, in_=inv)

            def gram_pre(blk):
                nonlocal zb_t
                gi, grp = pg_of[blk]
                if blk == grp[0]:
                    zb_t = zbpool.tile(
                        [128, len(grp) * FP * FP], fp32, tag="Zb"
                    )

            def gram_post(blk):
                gi, grp = pg_of[blk]
                if blk != grp[-1]:
                    return
                npk = len(grp)
                zbp = zb_t.rearrange("p (t m n) -> p t m n", t=npk, n=FP)
                Pk = pkpool.tile([128, npk * NPAIR], fp32, tag="Pk")
                pkp = Pk.rearrange("p (t c) -> p t c", t=npk)
                off = 0
                for m in range(NF - 1):
                    ln = NF - 1 - m
                    src = zbp[:, :, m, m + 1 : NF]
                    dst = pkp[:, :, off : off + ln]
                    if m % 2 == 0:
                        nc.vector.tensor_copy(out=dst, in_=src)
                    else:
                        nc.scalar.copy(dst, src)
                    off += ln
                b0 = grp[0] * BLK
                ovq = oview[b0 : b0 + npk * BLK].rearrange(
                    "(t q g) c -> g q t c", t=npk, g=4
                )
                pk4 = pkp.rearrange("(g q) t c -> g q t c", g=4)
                for g in range(4):
                    eng = nc.sync if g % 2 == 0 else nc.scalar
                    eng.dma_start(out=ovq[g], in_=pk4[g])

            # Pipeline: loads prefetch 2 blocks ahead of the transposes;
            # gram matmuls lag the transposes by one block so the PE queue
            # always has a gram burst banked behind a possibly-input-
            # stalled transpose burst.
            def emit_gram_block(blk):
                gram_pre(blk)
                emit_gram_half(blk, 0)
                emit_gram_half(blk, 1)
                gram_post(blk)

            for blk in range(5):
                emit_load(blk)
            emit_transpose_half(0, 0)
            emit_transpose_half(0, 1)
            emit_transpose_half(1, 0)
            emit_transpose_half(1, 1)
            emit_gram_block(0)
            emit_gram_block(1)
            for blk in range(2, nblk):
                if blk + 3 < nblk:
                    emit_load(blk + 3)
                emit_transpose_half(blk, 0)
                emit_transpose_half(blk, 1)
                if blk >= 3:
                    emit_gram_block(blk - 1)
            emit_gram_block(nblk - 1)

    nc.compile()
    return nc


def _get(bc=BC):
    if bc not in _CACHE:
        _CACHE[bc] = build(bc)
    return _CACHE[bc]


def kernel(dense: np.ndarray, embs: np.ndarray) -> np.ndarray:
    from concourse import bass_utils

    dense = np.ascontiguousarray(np.asarray(dense, dtype=np.float32))
    embs = np.ascontiguousarray(np.asarray(embs, dtype=np.float32))
    assert dense.shape == (B, D) and embs.shape == (B, NUM_EMBS, D)

    nc = _get()
    dsh = dense.reshape(N_CORES, BC, D)
    esh = embs.reshape(N_CORES, BC, NUM_EMBS, D)
    in_maps = [{"dense": dsh[i], "embs": esh[i]} for i in range(N_CORES)]
    res = bass_utils.run_bass_kernel_spmd(nc, in_maps, core_ids=list(range(N_CORES)))
    return np.concatenate([r["out"] for r in res.results], axis=0)
